# revision 37
# baseline (speedup 1.0000x reference)
"""Bidirectional-GRU document encoder (BiGRU + additive attention pooling)
for Trainium2, SPMD over 8 NeuronCores.

Sharding v2 — time-split: 8 cores = 2 directions x 2 doc-groups (16 docs)
x 2 time-halves.  Each core runs 320 local steps: 64 prefix steps + 256
kept steps covering its global-time half.  "Early" cores (which start at
the true sequence end for their direction) get an exact frozen prefix:
the host feeds warmup inputs x* solved so W_z x* + b_z ~ +40, making
z = sigmoid(40) = 1.0 exactly in fp32, so h' = z*h stays exactly 0
through the prefix.  "Late" cores warm up on the 64 real inputs
preceding their half; the h-seed error decays like prod(z) over 64
steps (~1e-3), far inside tolerance.  Serial depth drops 512 -> 320.

The per-step gate chain: 27 W_hh matmul pairs in r,z,n blocks with
separate one-bank PSUM tiles per gate (dep tracking is per-tile), a
single-matmul seed per gate from t-major xw layouts, and the
4-instruction tail sigmoid_r -> scan1 -> tanh -> scan2 via
TensorTensorScan.  q = sigmoid(-ghz) on ACT; GpSimd stays empty of
chain work so collective triggers cannot jam its queue.  Issue order is
dependency precision: consumers issue directly after their producer
block so coalesced semaphore waits snap to the true producer.

Exchange: pairwise AllGathers (fwd-half <-> bwd-core owning the same
global-time half) staged in thirds (B at kept 128, M at kept 192, A at
end) and resolved in 8-step sub-slices in the front slot of each step.
Attention scores for all kept chunks run in the tail (chunk 7 under the
final AllGather); pooling is partial (unnormalized exp) with per-core
partial sums combined on the host.
"""

import numpy as np
import ml_dtypes

import concourse.bacc as bacc
import concourse.bass as bass
import concourse.mybir as mybir
import concourse.tile as tile
from concourse.bass_utils import run_bass_kernel_spmd

F32 = mybir.dt.float32
BF16 = mybir.dt.bfloat16
AF = mybir.ActivationFunctionType
ALU = mybir.AluOpType
bf16 = ml_dtypes.bfloat16

# Problem constants
B, S, D, H = 32, 512, 768, 384
NCORES = 8
BG = 16                # docs per core
WARM = 64              # prefix steps per core
LSTEPS = S // 2 + WARM  # 320 local steps
KD = D // 128          # 6  k-chunks of input dim
M3 = 3 * H // 128      # 9  m-chunks of gate dim
KH = H // 128          # 3  k-chunks of hidden dim
MA = 2 * H // 128      # 6  m-chunks of attention rows


def build_program(steps=LSTEPS, bg=BG):
    """Build the SPMD Bass program (identical on all 8 cores)."""
    nc = bacc.Bacc("TRN2", target_bir_lowering=False, debug=False,
                   num_devices=NCORES)

    warm = WARM
    kept = steps - warm                     # 256 kept steps
    cols = steps * bg                       # P1 (t, b) plane incl. prefix
    ncol = 512                              # matmul N-chunk (one psum bank)
    pchunks = cols // ncol                  # 10 P1 chunks (local t)
    ct = ncol // bg                         # 32 timesteps per chunk
    achunks = kept * bg // ncol             # 8 attention chunks (kept t)
    qrt = kept // 4                         # 64
    half = kept // 2                        # 128

    # ---- DRAM I/O ----
    xt_d = nc.dram_tensor("xt", [KD, 128, cols], BF16, kind="ExternalInput")
    wih_d = nc.dram_tensor("wih", [M3 * KD, 128, 128], BF16, kind="ExternalInput")
    whh_d = nc.dram_tensor("whh", [M3 * KH, 128, 128], BF16, kind="ExternalInput")
    xwb_d = nc.dram_tensor("xwb", [128, M3], F32, kind="ExternalInput")
    idn_d = nc.dram_tensor("idn", [128, 128], BF16, kind="ExternalInput")
    bnb_d = nc.dram_tensor("bnb", [128, KH, bg, 2], BF16, kind="ExternalInput")
    wao_d = nc.dram_tensor("wao", [MA * KH, 128, 128], BF16, kind="ExternalInput")
    wap_d = nc.dram_tensor("wap", [MA * KH, 128, 128], BF16, kind="ExternalInput")
    bat_d = nc.dram_tensor("bat", [128, MA], F32, kind="ExternalInput")
    ctx_d = nc.dram_tensor("ctx", [128, MA], BF16, kind="ExternalInput")
    doc_d = nc.dram_tensor("doc", [128, KH, bg], F32, kind="ExternalOutput")
    ssum_d = nc.dram_tensor("ssum", [bg, 1], F32, kind="ExternalOutput")

    # Internal DRAM: hidden-state exchange staged in thirds.
    nA = qrt               # final exchange: u in [0, qrt)
    nB = kept - half       # first exchange: u in [half, kept)
    nM = half - qrt
    cc_inA = nc.dram_tensor("cc_inA", [128, nA, KH, bg], BF16)
    cc_outA = nc.dram_tensor("cc_outA", [2, 128, nA, KH, bg], BF16)
    cc_inB = nc.dram_tensor("cc_inB", [128, nB, KH, bg], BF16)
    cc_outB = nc.dram_tensor("cc_outB", [2, 128, nB, KH, bg], BF16)
    cc_inM = nc.dram_tensor("cc_inM", [128, nM, KH, bg], BF16)
    cc_outM = nc.dram_tensor("cc_outM", [2, 128, nM, KH, bg], BF16)
    sc_d = nc.dram_tensor("sc_scratch", [1, achunks, ct, bg], F32)
    at_d = nc.dram_tensor("at_scratch", [bg, kept], BF16)
    # pairs: (fwd-early-g, bwd-late-g) and (fwd-late-g, bwd-early-g)
    groups = [[0, 4], [1, 5], [2, 6], [3, 7]]

    # P1 pieces interleaved into the recurrence, globally greedy-packed:
    # each (c, m, khalf) piece lands on a free step before chunk c is
    # consumed (deadline c*ct - 2).
    pieces = {}
    upfront = 1
    for c in range(upfront, pchunks):
        lo = max(0, (c - 2) * ct)
        hi = c * ct - 2
        npc = 2 * M3
        for i in range(npc):
            s = lo + (i * (hi - lo)) // npc
            pieces.setdefault(s, []).append((c, i // 2, i % 2))

    with tile.TileContext(nc) as tc:
        with (
            tc.tile_pool(name="const", bufs=1) as cpool,
            tc.tile_pool(name="state", bufs=1) as spool,
            tc.tile_pool(name="work", bufs=2) as wpool,
        ):
            # ---- constants to SBUF ----
            whh = cpool.tile([128, M3 * KH, 128], BF16)
            xwb = cpool.tile([128, M3], F32)
            idn = cpool.tile([128, 128], BF16)
            bnb2 = cpool.tile([128, KH, bg, 2], BF16)
            wao = cpool.tile([128, MA * KH, 128], BF16)
            wap = cpool.tile([128, MA * KH, 128], BF16)
            bat = cpool.tile([128, MA], F32)
            ctxt = cpool.tile([128, MA], BF16)
            nc.sync.dma_start(whh[:], whh_d[:].rearrange("t p c -> p t c"))
            nc.sync.dma_start(xwb[:], xwb_d[:])
            nc.sync.dma_start(idn[:], idn_d[:])
            nc.sync.dma_start(bnb2[:], bnb_d[:])

            # ---- persistent state ----
            hist16 = spool.tile([128, KH, kept + 1, bg], BF16)
            # gate-chain scratch: pairwise-interleaved scan operands
            sb0 = spool.tile([128, 3, bg, 2], F32)    # parity1 <- sigmoid(r)
            zbuf = spool.tile([128, KH, bg], F32)     # sigmoid(z)
            nin2 = spool.tile([128, KH, bg, 2], F32)  # scan1 out; parity1 = nin
            bufn = spool.tile([128, KH, bg, 2], F32)  # parity1 <- tanh (n)
            bufq = spool.tile([128, KH, bg, 2], F32)  # parity0 <- q, parity1 <- zh
            ring0 = spool.tile([128, KH, bg, 2], BF16)
            ring1 = spool.tile([128, KH, bg, 2], BF16)
            ring = [ring0, ring1]                     # parity1 = h(t), ping-pong
            peer_hi = spool.tile([128, kept - qrt, KH, bg], BF16)
            nc.vector.memset(sb0[:], 0.0)             # zeros at parity 0
            nc.vector.memset(bufn[:], 0.0)
            nc.vector.memset(ring[1][:], 0.0)         # h(-1) = 0

            with (
                tc.tile_pool(name="xwp", bufs=1) as xwp,
                tc.tile_pool(name="xin", bufs=1) as xpool,
                tc.tile_pool(name="xtc", bufs=2) as xtp,
                tc.tile_pool(name="ps1", bufs=2,
                             space=bass.MemorySpace.PSUM) as psA,
                tc.tile_pool(name="psr", bufs=1,
                             space=bass.MemorySpace.PSUM) as psR,
                tc.tile_pool(name="psz", bufs=1,
                             space=bass.MemorySpace.PSUM) as psZ,
                tc.tile_pool(name="psn", bufs=1,
                             space=bass.MemorySpace.PSUM) as psN,
            ):
                # input projections, seed-friendly t-major layouts
                xws = xwp.tile([128, steps, 6, bg], BF16)
                xwn = xwp.tile([128, steps, KH, bg], BF16)
                wih = xpool.tile([128, M3 * KD, 128], BF16)
                nc.sync.dma_start(wih[:], wih_d[:].rearrange("t p c -> p t c"))

                xtc_tiles = {}
                px_tiles = {}
                pending_bias = []

                def xw_fetch(c):
                    csl = slice(c * ncol, (c + 1) * ncol)
                    xtc = xtp.tile([128, KD, ncol], BF16, tag="xtc")
                    for k in range(KD):
                        nc.sync.dma_start(xtc[:, k, :], xt_d[k][:, csl])
                    xtc_tiles[c] = xtc

                def xw_bias(c, m):
                    px = px_tiles.pop((c, m))
                    dst = (xws[:, c * ct:(c + 1) * ct, m, :] if m < 6
                           else xwn[:, c * ct:(c + 1) * ct, m - 6, :])
                    nc.vector.tensor_scalar(
                        out=dst,
                        in0=px[:].rearrange("p (t b) -> p t b", b=bg),
                        scalar1=xwb[:, m:m + 1],
                        scalar2=None, op0=ALU.add)

                def xw_piece(c, m, ks, defer=False):
                    if c not in xtc_tiles:
                        xw_fetch(c)
                    xtc = xtc_tiles[c]
                    if ks[0] == 0:
                        px = psA.tile([128, ncol], F32, tag="px")
                        px_tiles[(c, m)] = px
                    px = px_tiles[(c, m)]
                    for k in ks:
                        nc.tensor.matmul(
                            px[:], wih[:, m * KD + k, :], xtc[:, k, :],
                            start=(k == 0), stop=(k == KD - 1))
                    if ks[-1] == KD - 1:
                        if defer:
                            pending_bias.append((c, m))
                        else:
                            xw_bias(c, m)

                # Phase 1 prologue: chunk 0 so the recurrence can start
                for c in range(upfront):
                    for m in range(M3):
                        xw_piece(c, m, [0, 1, 2])
                        xw_piece(c, m, [3, 4, 5])

                # ======= Phase 2: GRU recurrence =======
                def seed(t):
                    gr = psR.tile([128, 3, bg], F32, tag="gr")
                    gz = psZ.tile([128, 3, bg], F32, tag="gz")
                    gn2 = psN.tile([128, KH, bg, 2], F32, tag="gn2")
                    nc.tensor.matmul(gr[:], idn[:], xws[:, t, 0:3, :],
                                     start=True, stop=False)
                    nc.tensor.matmul(gz[:], idn[:], xws[:, t, 3:6, :],
                                     start=True, stop=False)
                    nc.tensor.matmul(gn2[:], idn[:], bnb2[:],
                                     start=True, stop=False)
                    nc.tensor.matmul(gn2[:, :, :, 1], idn[:],
                                     xwn[:, t, :, :],
                                     start=False, stop=False)
                    return gr, gz, gn2

                RS = 4
                def resolve_slice(cout, cin, j0, w, j):
                    lo = w * ct + j * RS
                    sl = slice(lo, lo + RS)
                    pslice = peer_hi[:, j0 + lo:j0 + lo + RS, :, :]
                    rs1 = wpool.tile([128, RS, KH, bg], BF16, tag="rs1")
                    rso = wpool.tile([128, RS, KH, bg], BF16, tag="rso")
                    nc.sync.dma_start(pslice, cout[0][:, sl, :, :])
                    nc.sync.dma_start(rs1[:], cout[1][:, sl, :, :])
                    nc.sync.dma_start(rso[:], cin[:, sl, :, :])
                    nc.vector.tensor_tensor(out=pslice, in0=pslice,
                                            in1=rs1[:], op=ALU.add)
                    nc.vector.tensor_tensor(out=pslice, in0=pslice,
                                            in1=rso[:], op=ALU.subtract)

                front = {}
                spw = ct // RS
                for w in range(nB // ct):
                    for j in range(spw):
                        front.setdefault(warm + half + 17 + spw * w + j,
                                         []).append(
                            (cc_outB, cc_inB, half - qrt, w, j))
                for w in range(nM // ct):
                    for j in range(spw):
                        front.setdefault(warm + 3 * qrt + 17 + spw * w + j,
                                         []).append(
                            (cc_outM, cc_inM, 0, w, j))

                # attention weights are tail-only: load them behind the
                # recurrence, off the critical prologue DMA window
                nc.sync.dma_start(wao[:], wao_d[:].rearrange("t p c -> p t c"))
                nc.sync.dma_start(wap[:], wap_d[:].rearrange("t p c -> p t c"))
                nc.sync.dma_start(bat[:], bat_d[:])
                nc.sync.dma_start(ctxt[:], ctx_d[:])

                nxt = seed(0)
                for t in range(steps):
                    rcur = ring[t % 2]
                    rprev = ring[(t + 1) % 2]
                    gr, gz, gn2 = nxt
                    for m in (0, 1, 2):
                        for k in range(KH):
                            nc.tensor.matmul(
                                gr[:, m, :], whh[:, m * KH + k, :],
                                rprev[:, k, :, 1], start=False,
                                stop=(k == KH - 1 and m == 2))
                    # ---- front slot: previous step's bookkeeping ----
                    while pending_bias:
                        xw_bias(*pending_bias.pop(0))
                    if t > warm:
                        kc = t - warm          # hist col for h(t-1)
                        nc.vector.tensor_copy(hist16[:, :, kc, :],
                                              rprev[:, :, :, 1])
                        u = kept - kc          # = kept-1 - (kc-1)
                        if u >= half:
                            nc.sync.dma_start(cc_inB[:, u - half, :, :],
                                              hist16[:, :, kc, :])
                        elif u >= qrt:
                            nc.sync.dma_start(cc_inM[:, u - qrt, :, :],
                                              hist16[:, :, kc, :])
                        else:
                            nc.sync.dma_start(cc_inA[:, u, :, :],
                                              hist16[:, :, kc, :])
                    for args in front.get(t, ()):
                        resolve_slice(*args)
                    if t == warm + half:
                        nc.gpsimd.collective_compute(
                            "AllGather", ALU.bypass, replica_groups=groups,
                            ins=[cc_inB[:]], outs=[cc_outB[:]])
                    if t == warm + 3 * qrt:
                        nc.gpsimd.collective_compute(
                            "AllGather", ALU.bypass, replica_groups=groups,
                            ins=[cc_inM[:]], outs=[cc_outM[:]])
                    # r = sigmoid(ghr): waits only on the r-stop
                    nc.scalar.activation(sb0[:, :, :, 1], gr[:], AF.Sigmoid)
                    for m in (3, 4, 5):
                        for k in range(KH):
                            nc.tensor.matmul(
                                gz[:, m - 3, :], whh[:, m * KH + k, :],
                                rprev[:, k, :, 1], start=False,
                                stop=(k == KH - 1 and m == 5))
                    # z and q = sigmoid(+-ghz) on ACT; GpSimd stays empty
                    nc.scalar.activation(zbuf[:], gz[:], AF.Sigmoid)
                    nc.scalar.activation(bufq[:, :, :, 0], gz[:],
                                         AF.Sigmoid, scale=-1.0)
                    for m in (6, 7, 8):
                        for k in range(KH):
                            nc.tensor.matmul(
                                gn2[:, m - 6, :, 0], whh[:, m * KH + k, :],
                                rprev[:, k, :, 1], start=False,
                                stop=(k == KH - 1 and m == 8))
                    # scan1: even -> ghn, odd -> r*ghn + xn  (= nin)
                    nc.vector.tensor_tensor_scan(
                        out=nin2[:].rearrange("p c b j -> p (c b j)"),
                        data0=sb0[:].rearrange("p c b j -> p (c b j)"),
                        data1=gn2[:].rearrange("p c b j -> p (c b j)"),
                        initial=0.0, op0=ALU.mult, op1=ALU.add)
                    # zh = z * h(t-1): DVE, after scan1, before tanh's issue
                    nc.vector.tensor_tensor(
                        out=bufq[:, :, :, 1], in0=zbuf[:],
                        in1=rprev[:, :, :, 1], op=ALU.mult)
                    if t + 1 < steps:
                        nxt = seed(t + 1)
                    # n = tanh(nin) -> parity-1 of bufn
                    nc.scalar.activation(bufn[:, :, :, 1],
                                         nin2[:, :, :, 1], AF.Tanh)
                    # PE fill work in the idle window
                    for (c, m, kh) in pieces.get(t, ()):
                        xw_piece(c, m, [0, 1, 2] if kh == 0 else [3, 4, 5],
                                 defer=True)
                    # scan2: even -> q, odd -> n*q + zh  (= h')
                    nc.vector.tensor_tensor_scan(
                        out=rcur[:].rearrange("p c b j -> p (c b j)"),
                        data0=bufn[:].rearrange("p c b j -> p (c b j)"),
                        data1=bufq[:].rearrange("p c b j -> p (c b j)"),
                        initial=0.0, op0=ALU.mult, op1=ALU.add)

                # flush the last kept step
                rlast = ring[(steps - 1) % 2]
                nc.vector.tensor_copy(hist16[:, :, kept, :],
                                      rlast[:, :, :, 1])
                nc.sync.dma_start(cc_inA[:, 0, :, :], hist16[:, :, kept, :])

            # ======= Phase 3: exchange + attention + partial pooling =======
            ps3 = tc.tile_pool(name="ps3", bufs=1, space=bass.MemorySpace.PSUM)
            psA3 = ps3.__enter__()
            ps3b = tc.tile_pool(name="ps3b", bufs=2,
                                space=bass.MemorySpace.PSUM)
            psB3 = ps3b.__enter__()
            p3s = tc.tile_pool(name="p3s", bufs=1)
            spool3 = p3s.__enter__()
            p3w = tc.tile_pool(name="p3w", bufs=1)
            wpool3 = p3w.__enter__()

            nc.gpsimd.collective_compute(
                "AllGather", ALU.bypass, replica_groups=groups,
                ins=[cc_inA[:]], outs=[cc_outA[:]])
            peer_lo = spool3.tile([128, nA, KH, bg], BF16)

            own_pas = {}

            def chunk_own(nci):
                pas = []
                for m in range(MA):
                    pa = psA3.tile([128, ncol], F32, tag=f"pa{m}")
                    for k in range(KH):
                        nc.tensor.matmul(
                            pa[:], wao[:, m * KH + k, :],
                            hist16[:, k, 1 + nci * ct:1 + (nci + 1) * ct, :],
                            start=(k == 0), stop=False)
                    pas.append(pa)
                own_pas[nci] = pas

            def chunk_scores(nci, ptile, soff):
                tsl = slice(nci * ct - soff, (nci + 1) * ct - soff)
                psc = psB3.tile([1, ncol], F32, tag="psc")
                if nci not in own_pas:
                    chunk_own(nci)
                pas = own_pas.pop(nci)
                for m in range(MA):
                    for k in range(KH):
                        nc.tensor.matmul(
                            pas[m][:], wap[:, m * KH + k, :],
                            ptile[:, tsl, k, :],
                            start=False, stop=(k == KH - 1))
                ths = []
                for m in range(MA):
                    th = wpool3.tile([128, ncol], BF16, tag=f"th{m}")
                    nc.scalar.activation(th[:], pas[m][:], AF.Tanh,
                                         bias=bat[:, m:m + 1])
                    ths.append(th)
                for m in range(MA):
                    nc.tensor.matmul(psc[:], ctxt[:, m:m + 1], ths[m][:],
                                     start=(m == 0), stop=(m == MA - 1))
                scev = wpool3.tile([1, ncol], F32, tag="scev")
                nc.vector.tensor_copy(scev[:], psc[:])
                nc.sync.dma_start(
                    sc_d[0, nci].unsqueeze(0),
                    scev[:].rearrange("o (t b) -> o t b", t=ct))

            # chunk 7 + hi chunks (peer_hi resident) run during AllGather A
            for nci in range(2, achunks):
                chunk_scores(nci, peer_hi, qrt)

            def resolve_lo_slice(w):
                sl = slice(w * ct, (w + 1) * ct)
                pslice = peer_lo[:, sl, :, :]
                s1t = wpool3.tile([128, ct, KH, bg], BF16, tag="s1")
                ownr = wpool3.tile([128, ct, KH, bg], BF16, tag="ownr")
                nc.sync.dma_start(pslice, cc_outA[0][:, sl, :, :])
                nc.sync.dma_start(s1t[:], cc_outA[1][:, sl, :, :])
                nc.sync.dma_start(ownr[:], cc_inA[:, sl, :, :])
                nc.vector.tensor_tensor(out=pslice, in0=pslice, in1=s1t[:],
                                        op=ALU.add)
                nc.vector.tensor_tensor(out=pslice, in0=pslice, in1=ownr[:],
                                        op=ALU.subtract)

            for w, nci in enumerate(range(2)):
                chunk_own(nci)       # own-half mms hide the resolve
                resolve_lo_slice(w)
                chunk_scores(nci, peer_lo, 0)

            # scores -> [bg, kept] via DRAM; unnormalized softmax weights
            sc = spool3.tile([bg, kept], F32)
            nc.sync.dma_start(sc[:].rearrange("b (n t) -> b n t", n=achunks),
                              sc_d[0].rearrange("n t b -> b n t"))
            esc = wpool3.tile([bg, kept], F32, tag="esc")
            ssum = wpool3.tile([bg, 1], F32, tag="ssum")
            nc.scalar.activation(esc[:], sc[:], AF.Exp, accum_out=ssum[:])
            nc.sync.dma_start(ssum_d[:], ssum[:])
            attn = spool3.tile([bg, kept], BF16)
            nc.vector.tensor_copy(attn[:], esc[:])
            # broadcast attn to all partitions as [128, (b, t)] via DRAM
            nc.sync.dma_start(at_d[:], attn[:])
            attn_bc = spool3.tile([128, bg, kept], BF16)
            nc.sync.dma_start(attn_bc[:],
                              at_d[:].unsqueeze(0).broadcast_to(
                                  [128, bg, kept]))

            # partial pooling: P[p, c, b] = sum_t h[p, c, t, b] * e[b, t]
            doc = spool3.tile([128, KH, bg], F32)
            with tc.tile_pool(name="poolw", bufs=1) as ppool:
                for c in range(KH):
                    wprod = ppool.tile([128, bg, kept], BF16, tag="wprod")
                    nc.vector.tensor_tensor(
                        out=wprod[:],
                        in0=hist16[:, c, 1:, :].rearrange("p t b -> p b t"),
                        in1=attn_bc[:], op=ALU.mult)
                    nc.vector.reduce_sum(doc[:, c, :], wprod[:],
                                         axis=mybir.AxisListType.X)
            nc.sync.dma_start(doc_d[:], doc[:])
            p3w.__exit__(None, None, None)
            p3s.__exit__(None, None, None)
            ps3b.__exit__(None, None, None)
            ps3.__exit__(None, None, None)

    nc.compile()
    return nc


def _tiles(w, kc, mc):
    """w: [kc*128, mc*128] -> [mc*kc, 128, 128] lhsT tiles, m-major."""
    out = np.empty((mc * kc, 128, 128), dtype=w.dtype)
    for m in range(mc):
        for k in range(kc):
            out[m * kc + k] = w[k * 128:(k + 1) * 128, m * 128:(m + 1) * 128]
    return out


def _freeze_input(W_ih):
    """Least-norm x* with W_z x* = +40: z = sigmoid(40) == 1.0 in fp32."""
    Wz = np.asarray(W_ih[H:2 * H], np.float64)
    rhs = np.full(H, 40.0)
    x = Wz.T @ np.linalg.solve(Wz @ Wz.T, rhs)
    return x.astype(np.float32)


def host_prep(inputs, steps=LSTEPS, bg=BG):
    """Build the 8 per-core input maps (all host-side numpy)."""
    ip = np.asarray(inputs["ip"], np.float32)
    W_attn = np.asarray(inputs["W_attn"], np.float32)
    b_attn = np.asarray(inputs["b_attn"], np.float32)
    ctx = np.asarray(inputs["context"], np.float32)
    warm = WARM
    kept = steps - warm
    maps = []
    for core in range(NCORES):
        fwd = core < 4
        late = (core % 4 >= 2) if fwd else (core % 4 < 2)
        g = core % 2
        sfx = "f" if fwd else "b"
        W_ih = np.asarray(inputs[f"W_ih_{sfx}"], np.float32)
        W_hh = np.asarray(inputs[f"W_hh_{sfx}"], np.float32)
        b_ih = np.asarray(inputs[f"b_ih_{sfx}"], np.float32)
        b_hh = np.asarray(inputs[f"b_hh_{sfx}"], np.float32)

        xg = ip[g * bg:(g + 1) * bg]             # [bg, S, D]
        if not fwd:
            xg = xg[:, ::-1, :]                  # processes tau = S-1 .. 0
        # local window: early = frozen prefix + first kept half;
        # late = real warmup prefix + second kept half
        if late:
            x = xg[:, kept - warm:, :]           # [bg, warm+kept, D]
        else:
            xf = np.broadcast_to(_freeze_input(W_ih), (bg, warm, D))
            x = np.concatenate([xf, xg[:, :kept, :]], axis=1)
        assert x.shape[1] == steps

        xt = np.ascontiguousarray(x.transpose(2, 1, 0))     # [D, steps, bg]
        xt = xt.reshape(KD, 128, steps * bg)
        bias = b_ih + np.concatenate([b_hh[:2 * H], np.zeros(H, np.float32)])
        own = slice(0, H) if fwd else slice(H, 2 * H)
        pr = slice(H, 2 * H) if fwd else slice(0, H)
        bnb2 = np.zeros((128, KH, bg, 2), np.float32)
        bnb2[:, :, :, 0] = np.ascontiguousarray(
            b_hh[2 * H:].reshape(KH, 128).T)[:, :, None]
        m = {
            "xt": xt.astype(bf16),
            "wih": _tiles(W_ih.T.astype(bf16), KD, M3),
            "whh": _tiles(W_hh.T.astype(bf16), KH, M3),
            "xwb": np.ascontiguousarray(bias.reshape(M3, 128).T),
            "idn": np.eye(128, dtype=np.float32).astype(bf16),
            "bnb": bnb2.astype(bf16),
            "wao": _tiles(np.ascontiguousarray(W_attn[:, own].T).astype(bf16),
                          KH, MA),
            "wap": _tiles(np.ascontiguousarray(W_attn[:, pr].T).astype(bf16),
                          KH, MA),
            "bat": np.ascontiguousarray(b_attn.reshape(MA, 128).T),
            "ctx": np.ascontiguousarray(ctx.reshape(MA, 128).T).astype(bf16),
        }
        maps.append(m)
    return maps


def assemble(results, steps=LSTEPS, bg=BG):
    """Combine per-core partial pools: doc = sum(P)/sum(S) per half."""
    doc = np.zeros((B, 2 * H), np.float32)
    for dir_ in range(2):
        for g in range(2):
            early = g if dir_ == 0 else 6 + g
            late = 2 + g if dir_ == 0 else 4 + g
            Pe = np.asarray(results[early]["doc"]).transpose(2, 1, 0)
            Pl = np.asarray(results[late]["doc"]).transpose(2, 1, 0)
            Se = np.asarray(results[early]["ssum"])[:, 0]
            Sl = np.asarray(results[late]["ssum"])[:, 0]
            comb = (Pe + Pl).reshape(bg, H) / (Se + Sl)[:, None]
            half = slice(0, H) if dir_ == 0 else slice(H, 2 * H)
            doc[g * bg:(g + 1) * bg, half] = comb
    return doc


def kernel(**inputs):
    nc = build_program(LSTEPS, BG)
    in_maps = host_prep(inputs, LSTEPS, BG)
    res = run_bass_kernel_spmd(nc, in_maps, list(range(NCORES)))
    return assemble(res.results, LSTEPS, BG)


# revision 38
# speedup vs baseline: 1.2051x; 1.2051x over previous
"""Bidirectional-GRU document encoder (BiGRU + additive attention pooling)
for Trainium2, SPMD over 8 NeuronCores.

Sharding v2 — time-split: 8 cores = 2 directions x 2 doc-groups (16 docs)
x 2 time-halves.  Each core runs 320 local steps: 64 prefix steps + 256
kept steps covering its global-time half.  "Early" cores (which start at
the true sequence end for their direction) get an exact frozen prefix:
the host feeds warmup inputs x* solved so W_z x* + b_z ~ +40, making
z = sigmoid(40) = 1.0 exactly in fp32, so h' = z*h stays exactly 0
through the prefix.  "Late" cores warm up on the 64 real inputs
preceding their half; the h-seed error decays like prod(z) over 64
steps (~1e-3), far inside tolerance.  Serial depth drops 512 -> 320.

The per-step gate chain: 27 W_hh matmul pairs in r,z,n blocks with
separate one-bank PSUM tiles per gate (dep tracking is per-tile), a
single-matmul seed per gate from t-major xw layouts, and the
4-instruction tail sigmoid_r -> scan1 -> tanh -> scan2 via
TensorTensorScan.  q = sigmoid(-ghz) on ACT; GpSimd stays empty of
chain work so collective triggers cannot jam its queue.  Issue order is
dependency precision: consumers issue directly after their producer
block so coalesced semaphore waits snap to the true producer.

Exchange: pairwise AllGathers (fwd-half <-> bwd-core owning the same
global-time half) staged in thirds (B at kept 128, M at kept 192, A at
end) and resolved in 8-step sub-slices in the front slot of each step.
Attention scores for all kept chunks run in the tail (chunk 7 under the
final AllGather); pooling is partial (unnormalized exp) with per-core
partial sums combined on the host.
"""

import numpy as np
import ml_dtypes

import concourse.bacc as bacc
import concourse.bass as bass
import concourse.mybir as mybir
import concourse.tile as tile
from concourse.bass_utils import run_bass_kernel_spmd

F32 = mybir.dt.float32
BF16 = mybir.dt.bfloat16
AF = mybir.ActivationFunctionType
ALU = mybir.AluOpType
bf16 = ml_dtypes.bfloat16

# Problem constants
B, S, D, H = 32, 512, 768, 384
NCORES = 8
BG = 16                # docs per core
WARM = 64              # prefix steps per core
LSTEPS = S // 2 + WARM  # 320 local steps
KD = D // 128          # 6  k-chunks of input dim
M3 = 3 * H // 128      # 9  m-chunks of gate dim
KH = H // 128          # 3  k-chunks of hidden dim
MA = 2 * H // 128      # 6  m-chunks of attention rows


def build_program(steps=LSTEPS, bg=BG):
    """Build the SPMD Bass program (identical on all 8 cores)."""
    nc = bacc.Bacc("TRN2", target_bir_lowering=False, debug=False,
                   num_devices=NCORES)

    warm = WARM
    kept = steps - warm                     # 256 kept steps
    cols = steps * bg                       # P1 (t, b) plane incl. prefix
    ncol = 512                              # matmul N-chunk (one psum bank)
    pchunks = cols // ncol                  # 10 P1 chunks (local t)
    ct = ncol // bg                         # 32 timesteps per chunk
    achunks = kept * bg // ncol             # 8 attention chunks (kept t)
    qrt = kept // 4                         # 64
    half = kept // 2                        # 128

    # ---- DRAM I/O ----
    xt_d = nc.dram_tensor("xt", [KD, 128, cols], BF16, kind="ExternalInput")
    wih_d = nc.dram_tensor("wih", [M3 * KD, 128, 128], BF16, kind="ExternalInput")
    whh_d = nc.dram_tensor("whh", [M3 * KH, 128, 128], BF16, kind="ExternalInput")
    xwb_d = nc.dram_tensor("xwb", [128, M3], F32, kind="ExternalInput")
    idn_d = nc.dram_tensor("idn", [128, 128], BF16, kind="ExternalInput")
    bnb_d = nc.dram_tensor("bnb", [128, KH, bg, 2], BF16, kind="ExternalInput")
    wao_d = nc.dram_tensor("wao", [MA * KH, 128, 128], BF16, kind="ExternalInput")
    wap_d = nc.dram_tensor("wap", [MA * KH, 128, 128], BF16, kind="ExternalInput")
    bat_d = nc.dram_tensor("bat", [128, MA], F32, kind="ExternalInput")
    ctx_d = nc.dram_tensor("ctx", [128, MA], BF16, kind="ExternalInput")
    doc_d = nc.dram_tensor("doc", [128, KH, bg], F32, kind="ExternalOutput")
    ssum_d = nc.dram_tensor("ssum", [bg, 1], F32, kind="ExternalOutput")

    # Internal DRAM: hidden-state exchange staged in thirds.
    nA = qrt               # final exchange: u in [0, qrt)
    nB = kept - half       # first exchange: u in [half, kept)
    nM = half - qrt
    cc_inA = nc.dram_tensor("cc_inA", [128, nA, KH, bg], BF16)
    cc_outA = nc.dram_tensor("cc_outA", [2, 128, nA, KH, bg], BF16)
    cc_inB = nc.dram_tensor("cc_inB", [128, nB, KH, bg], BF16)
    cc_outB = nc.dram_tensor("cc_outB", [2, 128, nB, KH, bg], BF16)
    cc_inM = nc.dram_tensor("cc_inM", [128, nM, KH, bg], BF16)
    cc_outM = nc.dram_tensor("cc_outM", [2, 128, nM, KH, bg], BF16)
    sc_d = nc.dram_tensor("sc_scratch", [1, achunks, ct, bg], F32)
    at_d = nc.dram_tensor("at_scratch", [bg, kept], BF16)
    # pairs: (fwd-early-g, bwd-late-g) and (fwd-late-g, bwd-early-g)
    groups = [[0, 4], [1, 5], [2, 6], [3, 7]]

    # P1 pieces interleaved into the recurrence, globally greedy-packed:
    # each (c, m, khalf) piece lands on a free step before chunk c is
    # consumed (deadline c*ct - 2).
    # attention chunks 4-6 injected into recurrence idle windows:
    # own h ready at warm+32*(nci+1)+1; peer rows resolved by ~warm+161
    inj_base = {4: warm + 162, 5: warm + 194, 6: warm + 226}
    pieces = {}
    upfront = 1
    for c in range(upfront, pchunks):
        lo = max(0, (c - 2) * ct)
        hi = c * ct - 2
        npc = 2 * M3
        for i in range(npc):
            s = lo + (i * (hi - lo)) // npc
            pieces.setdefault(s, []).append((c, i // 2, i % 2))

    with tile.TileContext(nc) as tc:
        with (
            tc.tile_pool(name="const", bufs=1) as cpool,
            tc.tile_pool(name="state", bufs=1) as spool,
            tc.tile_pool(name="work", bufs=2) as wpool,
        ):
            # ---- constants to SBUF ----
            whh = cpool.tile([128, M3 * KH, 128], BF16)
            xwb = cpool.tile([128, M3], F32)
            idn = cpool.tile([128, 128], BF16)
            bnb2 = cpool.tile([128, KH, bg, 2], BF16)
            wao = cpool.tile([128, MA * KH, 128], BF16)
            wap = cpool.tile([128, MA * KH, 128], BF16)
            bat = cpool.tile([128, MA], F32)
            ctxt = cpool.tile([128, MA], BF16)
            nc.sync.dma_start(whh[:], whh_d[:].rearrange("t p c -> p t c"))
            nc.sync.dma_start(xwb[:], xwb_d[:])
            nc.sync.dma_start(idn[:], idn_d[:])
            nc.sync.dma_start(bnb2[:], bnb_d[:])

            # ---- persistent state ----
            hist16 = spool.tile([128, KH, kept + 1, bg], BF16)
            # gate-chain scratch: pairwise-interleaved scan operands
            sb0 = spool.tile([128, 3, bg, 2], F32)    # parity1 <- sigmoid(r)
            zbuf = spool.tile([128, KH, bg], F32)     # sigmoid(z)
            nin2 = spool.tile([128, KH, bg, 2], F32)  # scan1 out; parity1 = nin
            bufn = spool.tile([128, KH, bg, 2], F32)  # parity1 <- tanh (n)
            bufq = spool.tile([128, KH, bg, 2], F32)  # parity0 <- q, parity1 <- zh
            ring0 = spool.tile([128, KH, bg, 2], BF16)
            ring1 = spool.tile([128, KH, bg, 2], BF16)
            ring = [ring0, ring1]                     # parity1 = h(t), ping-pong
            peer_hi = spool.tile([128, kept - qrt, KH, bg], BF16)
            nc.vector.memset(sb0[:], 0.0)             # zeros at parity 0
            nc.vector.memset(bufn[:], 0.0)
            nc.vector.memset(ring[1][:], 0.0)         # h(-1) = 0

            with (
                tc.tile_pool(name="xwp", bufs=1) as xwp,
                tc.tile_pool(name="xin", bufs=1) as xpool,
                tc.tile_pool(name="xtc", bufs=2) as xtp,
                tc.tile_pool(name="ps1", bufs=2,
                             space=bass.MemorySpace.PSUM) as psA,
                tc.tile_pool(name="psr", bufs=1,
                             space=bass.MemorySpace.PSUM) as psR,
                tc.tile_pool(name="psz", bufs=1,
                             space=bass.MemorySpace.PSUM) as psZ,
                tc.tile_pool(name="psn", bufs=1,
                             space=bass.MemorySpace.PSUM) as psN,
                tc.tile_pool(name="pssc", bufs=2,
                             space=bass.MemorySpace.PSUM) as psSC,
                tc.tile_pool(name="pscm", bufs=1,
                             space=bass.MemorySpace.PSUM) as psCM,
            ):
                # input projections, seed-friendly t-major layouts
                xws = xwp.tile([128, steps, 6, bg], BF16)
                xwn = xwp.tile([128, steps, KH, bg], BF16)
                wih = xpool.tile([128, M3 * KD, 128], BF16)
                nc.sync.dma_start(wih[:], wih_d[:].rearrange("t p c -> p t c"))

                xtc_tiles = {}
                px_tiles = {}
                pending_bias = []

                def xw_fetch(c):
                    csl = slice(c * ncol, (c + 1) * ncol)
                    xtc = xtp.tile([128, KD, ncol], BF16, tag="xtc")
                    for k in range(KD):
                        nc.sync.dma_start(xtc[:, k, :], xt_d[k][:, csl])
                    xtc_tiles[c] = xtc

                def xw_bias(c, m):
                    px = px_tiles.pop((c, m))
                    dst = (xws[:, c * ct:(c + 1) * ct, m, :] if m < 6
                           else xwn[:, c * ct:(c + 1) * ct, m - 6, :])
                    nc.vector.tensor_scalar(
                        out=dst,
                        in0=px[:].rearrange("p (t b) -> p t b", b=bg),
                        scalar1=xwb[:, m:m + 1],
                        scalar2=None, op0=ALU.add)

                def xw_piece(c, m, ks, defer=False):
                    if c not in xtc_tiles:
                        xw_fetch(c)
                    xtc = xtc_tiles[c]
                    if ks[0] == 0:
                        px = psA.tile([128, ncol], F32, tag="px")
                        px_tiles[(c, m)] = px
                    px = px_tiles[(c, m)]
                    for k in ks:
                        nc.tensor.matmul(
                            px[:], wih[:, m * KD + k, :], xtc[:, k, :],
                            start=(k == 0), stop=(k == KD - 1))
                    if ks[-1] == KD - 1:
                        if defer:
                            pending_bias.append((c, m))
                        else:
                            xw_bias(c, m)

                # Phase 1 prologue: chunk 0 so the recurrence can start
                for c in range(upfront):
                    for m in range(M3):
                        xw_piece(c, m, [0, 1, 2])
                        xw_piece(c, m, [3, 4, 5])

                # ======= Phase 2: GRU recurrence =======
                def seed(t):
                    gr = psR.tile([128, 3, bg], F32, tag="gr")
                    gz = psZ.tile([128, 3, bg], F32, tag="gz")
                    gn2 = psN.tile([128, KH, bg, 2], F32, tag="gn2")
                    nc.tensor.matmul(gr[:], idn[:], xws[:, t, 0:3, :],
                                     start=True, stop=False)
                    nc.tensor.matmul(gz[:], idn[:], xws[:, t, 3:6, :],
                                     start=True, stop=False)
                    nc.tensor.matmul(gn2[:], idn[:], bnb2[:],
                                     start=True, stop=False)
                    nc.tensor.matmul(gn2[:, :, :, 1], idn[:],
                                     xwn[:, t, :, :],
                                     start=False, stop=False)
                    return gr, gz, gn2

                RS = 4
                def resolve_slice(cout, cin, j0, w, j):
                    lo = w * ct + j * RS
                    sl = slice(lo, lo + RS)
                    pslice = peer_hi[:, j0 + lo:j0 + lo + RS, :, :]
                    rs1 = wpool.tile([128, RS, KH, bg], BF16, tag="rs1")
                    rso = wpool.tile([128, RS, KH, bg], BF16, tag="rso")
                    nc.sync.dma_start(pslice, cout[0][:, sl, :, :])
                    nc.sync.dma_start(rs1[:], cout[1][:, sl, :, :])
                    nc.sync.dma_start(rso[:], cin[:, sl, :, :])
                    nc.vector.tensor_tensor(out=pslice, in0=pslice,
                                            in1=rs1[:], op=ALU.add)
                    nc.vector.tensor_tensor(out=pslice, in0=pslice,
                                            in1=rso[:], op=ALU.subtract)

                front = {}
                spw = ct // RS
                for w in range(nB // ct):
                    for j in range(spw):
                        front.setdefault(warm + half + 17 + spw * w + j,
                                         []).append(
                            (cc_outB, cc_inB, half - qrt, w, j))
                for w in range(nM // ct):
                    for j in range(spw):
                        front.setdefault(warm + 3 * qrt + 17 + spw * w + j,
                                         []).append(
                            (cc_outM, cc_inM, 0, w, j))

                inj_state = {}

                def inj_own(nci, m):
                    pa = psSC.tile([128, ncol], F32, tag="spa")
                    inj_state[(nci, m)] = pa
                    if m == 0:
                        psc = psCM.tile([1, ncol], F32, tag="pscm")
                        inj_state[nci] = psc
                    for k in range(KH):
                        nc.tensor.matmul(
                            pa[:], wao[:, m * KH + k, :],
                            hist16[:, k, 1 + nci * ct:1 + (nci + 1) * ct, :],
                            start=(k == 0), stop=False)

                def inj_peer(nci, m):
                    pa = inj_state[(nci, m)]
                    s0 = nci * ct - qrt
                    for k in range(KH):
                        nc.tensor.matmul(
                            pa[:], wap[:, m * KH + k, :],
                            peer_hi[:, s0:s0 + ct, k, :],
                            start=False, stop=(k == KH - 1))

                def inj_tanh(nci, m):
                    pa = inj_state.pop((nci, m))
                    psc = inj_state[nci]
                    th = wpool.tile([128, ncol], BF16, tag="ith")
                    nc.scalar.activation(th[:], pa[:], AF.Tanh,
                                         bias=bat[:, m:m + 1])
                    nc.tensor.matmul(psc[:], ctxt[:, m:m + 1], th[:],
                                     start=(m == 0), stop=(m == MA - 1))

                def inj_done(nci):
                    psc = inj_state.pop(nci)
                    scev = wpool.tile([1, ncol], F32, tag="iscev")
                    nc.vector.tensor_copy(scev[:], psc[:])
                    nc.sync.dma_start(
                        sc_d[0, nci].unsqueeze(0),
                        scev[:].rearrange("o (t b) -> o t b", t=ct))

                inject = {}
                for nci, base in inj_base.items():
                    for m in range(MA):
                        inject.setdefault(base + 3 * m, []).append(
                            (inj_own, (nci, m)))
                        inject.setdefault(base + 3 * m + 1, []).append(
                            (inj_peer, (nci, m)))
                        inject.setdefault(base + 3 * m + 2, []).append(
                            (inj_tanh, (nci, m)))
                    inject.setdefault(base + 3 * MA, []).append(
                        (inj_done, (nci,)))

                # attention weights are tail-only: load them behind the
                # recurrence, off the critical prologue DMA window
                nc.sync.dma_start(wao[:], wao_d[:].rearrange("t p c -> p t c"))
                nc.sync.dma_start(wap[:], wap_d[:].rearrange("t p c -> p t c"))
                nc.sync.dma_start(bat[:], bat_d[:])
                nc.sync.dma_start(ctxt[:], ctx_d[:])

                nxt = seed(0)
                for t in range(steps):
                    rcur = ring[t % 2]
                    rprev = ring[(t + 1) % 2]
                    gr, gz, gn2 = nxt
                    for m in (0, 1, 2):
                        for k in range(KH):
                            nc.tensor.matmul(
                                gr[:, m, :], whh[:, m * KH + k, :],
                                rprev[:, k, :, 1], start=False,
                                stop=(k == KH - 1 and m == 2))
                    # ---- front slot: previous step's bookkeeping ----
                    while pending_bias:
                        xw_bias(*pending_bias.pop(0))
                    if t > warm:
                        kc = t - warm          # hist col for h(t-1)
                        nc.vector.tensor_copy(hist16[:, :, kc, :],
                                              rprev[:, :, :, 1])
                        u = kept - kc          # = kept-1 - (kc-1)
                        if u >= half:
                            nc.sync.dma_start(cc_inB[:, u - half, :, :],
                                              hist16[:, :, kc, :])
                        elif u >= qrt:
                            nc.sync.dma_start(cc_inM[:, u - qrt, :, :],
                                              hist16[:, :, kc, :])
                        else:
                            nc.sync.dma_start(cc_inA[:, u, :, :],
                                              hist16[:, :, kc, :])
                    for args in front.get(t, ()):
                        resolve_slice(*args)
                    if t == warm + half:
                        nc.gpsimd.collective_compute(
                            "AllGather", ALU.bypass, replica_groups=groups,
                            ins=[cc_inB[:]], outs=[cc_outB[:]])
                    if t == warm + 3 * qrt:
                        nc.gpsimd.collective_compute(
                            "AllGather", ALU.bypass, replica_groups=groups,
                            ins=[cc_inM[:]], outs=[cc_outM[:]])
                    # r = sigmoid(ghr): waits only on the r-stop
                    nc.scalar.activation(sb0[:, :, :, 1], gr[:], AF.Sigmoid)
                    for m in (3, 4, 5):
                        for k in range(KH):
                            nc.tensor.matmul(
                                gz[:, m - 3, :], whh[:, m * KH + k, :],
                                rprev[:, k, :, 1], start=False,
                                stop=(k == KH - 1 and m == 5))
                    # z and q = sigmoid(+-ghz) on ACT; GpSimd stays empty
                    nc.scalar.activation(zbuf[:], gz[:], AF.Sigmoid)
                    nc.scalar.activation(bufq[:, :, :, 0], gz[:],
                                         AF.Sigmoid, scale=-1.0)
                    for m in (6, 7, 8):
                        for k in range(KH):
                            nc.tensor.matmul(
                                gn2[:, m - 6, :, 0], whh[:, m * KH + k, :],
                                rprev[:, k, :, 1], start=False,
                                stop=(k == KH - 1 and m == 8))
                    # scan1: even -> ghn, odd -> r*ghn + xn  (= nin)
                    nc.vector.tensor_tensor_scan(
                        out=nin2[:].rearrange("p c b j -> p (c b j)"),
                        data0=sb0[:].rearrange("p c b j -> p (c b j)"),
                        data1=gn2[:].rearrange("p c b j -> p (c b j)"),
                        initial=0.0, op0=ALU.mult, op1=ALU.add)
                    # zh = z * h(t-1): DVE, after scan1, before tanh's issue
                    nc.vector.tensor_tensor(
                        out=bufq[:, :, :, 1], in0=zbuf[:],
                        in1=rprev[:, :, :, 1], op=ALU.mult)
                    if t + 1 < steps:
                        nxt = seed(t + 1)
                    # n = tanh(nin) -> parity-1 of bufn
                    nc.scalar.activation(bufn[:, :, :, 1],
                                         nin2[:, :, :, 1], AF.Tanh)
                    # PE fill work in the idle window
                    for (c, m, kh) in pieces.get(t, ()):
                        xw_piece(c, m, [0, 1, 2] if kh == 0 else [3, 4, 5],
                                 defer=True)
                    for fn, args in inject.get(t, ()):
                        if fn in (inj_own, inj_peer):
                            fn(*args)
                    # scan2: even -> q, odd -> n*q + zh  (= h')
                    nc.vector.tensor_tensor_scan(
                        out=rcur[:].rearrange("p c b j -> p (c b j)"),
                        data0=bufn[:].rearrange("p c b j -> p (c b j)"),
                        data1=bufq[:].rearrange("p c b j -> p (c b j)"),
                        initial=0.0, op0=ALU.mult, op1=ALU.add)
                    for fn, args in inject.get(t, ()):
                        if fn not in (inj_own, inj_peer):
                            fn(*args)

                # flush the last kept step
                rlast = ring[(steps - 1) % 2]
                nc.vector.tensor_copy(hist16[:, :, kept, :],
                                      rlast[:, :, :, 1])
                nc.sync.dma_start(cc_inA[:, 0, :, :], hist16[:, :, kept, :])

            # ======= Phase 3: exchange + attention + partial pooling =======
            ps3 = tc.tile_pool(name="ps3", bufs=1, space=bass.MemorySpace.PSUM)
            psA3 = ps3.__enter__()
            ps3b = tc.tile_pool(name="ps3b", bufs=2,
                                space=bass.MemorySpace.PSUM)
            psB3 = ps3b.__enter__()
            p3s = tc.tile_pool(name="p3s", bufs=1)
            spool3 = p3s.__enter__()
            p3w = tc.tile_pool(name="p3w", bufs=1)
            wpool3 = p3w.__enter__()

            nc.gpsimd.collective_compute(
                "AllGather", ALU.bypass, replica_groups=groups,
                ins=[cc_inA[:]], outs=[cc_outA[:]])
            peer_lo = spool3.tile([128, nA, KH, bg], BF16)

            own_pas = {}

            def chunk_own(nci):
                pas = []
                for m in range(MA):
                    pa = psA3.tile([128, ncol], F32, tag=f"pa{m}")
                    for k in range(KH):
                        nc.tensor.matmul(
                            pa[:], wao[:, m * KH + k, :],
                            hist16[:, k, 1 + nci * ct:1 + (nci + 1) * ct, :],
                            start=(k == 0), stop=False)
                    pas.append(pa)
                own_pas[nci] = pas

            def chunk_scores(nci, ptile, soff):
                tsl = slice(nci * ct - soff, (nci + 1) * ct - soff)
                psc = psB3.tile([1, ncol], F32, tag="psc")
                if nci not in own_pas:
                    chunk_own(nci)
                pas = own_pas.pop(nci)
                for m in range(MA):
                    for k in range(KH):
                        nc.tensor.matmul(
                            pas[m][:], wap[:, m * KH + k, :],
                            ptile[:, tsl, k, :],
                            start=False, stop=(k == KH - 1))
                ths = []
                for m in range(MA):
                    th = wpool3.tile([128, ncol], BF16, tag=f"th{m}")
                    nc.scalar.activation(th[:], pas[m][:], AF.Tanh,
                                         bias=bat[:, m:m + 1])
                    ths.append(th)
                for m in range(MA):
                    nc.tensor.matmul(psc[:], ctxt[:, m:m + 1], ths[m][:],
                                     start=(m == 0), stop=(m == MA - 1))
                scev = wpool3.tile([1, ncol], F32, tag="scev")
                nc.vector.tensor_copy(scev[:], psc[:])
                nc.sync.dma_start(
                    sc_d[0, nci].unsqueeze(0),
                    scev[:].rearrange("o (t b) -> o t b", t=ct))

            # chunk 7 + remaining hi chunks run during AllGather A
            for nci in range(2, achunks):
                if nci not in inj_base:
                    chunk_scores(nci, peer_hi, qrt)

            def resolve_lo_slice(w):
                sl = slice(w * ct, (w + 1) * ct)
                pslice = peer_lo[:, sl, :, :]
                s1t = wpool3.tile([128, ct, KH, bg], BF16, tag="s1")
                ownr = wpool3.tile([128, ct, KH, bg], BF16, tag="ownr")
                nc.sync.dma_start(pslice, cc_outA[0][:, sl, :, :])
                nc.sync.dma_start(s1t[:], cc_outA[1][:, sl, :, :])
                nc.sync.dma_start(ownr[:], cc_inA[:, sl, :, :])
                nc.vector.tensor_tensor(out=pslice, in0=pslice, in1=s1t[:],
                                        op=ALU.add)
                nc.vector.tensor_tensor(out=pslice, in0=pslice, in1=ownr[:],
                                        op=ALU.subtract)

            for w, nci in enumerate(range(2)):
                chunk_own(nci)       # own-half mms hide the resolve
                resolve_lo_slice(w)
                chunk_scores(nci, peer_lo, 0)

            # scores -> [bg, kept] via DRAM; unnormalized softmax weights
            sc = spool3.tile([bg, kept], F32)
            nc.sync.dma_start(sc[:].rearrange("b (n t) -> b n t", n=achunks),
                              sc_d[0].rearrange("n t b -> b n t"))
            esc = wpool3.tile([bg, kept], F32, tag="esc")
            ssum = wpool3.tile([bg, 1], F32, tag="ssum")
            nc.scalar.activation(esc[:], sc[:], AF.Exp, accum_out=ssum[:])
            nc.sync.dma_start(ssum_d[:], ssum[:])
            attn = spool3.tile([bg, kept], BF16)
            nc.vector.tensor_copy(attn[:], esc[:])
            # broadcast attn to all partitions as [128, (b, t)] via DRAM
            nc.sync.dma_start(at_d[:], attn[:])
            attn_bc = spool3.tile([128, bg, kept], BF16)
            nc.sync.dma_start(attn_bc[:],
                              at_d[:].unsqueeze(0).broadcast_to(
                                  [128, bg, kept]))

            # partial pooling: P[p, c, b] = sum_t h[p, c, t, b] * e[b, t]
            doc = spool3.tile([128, KH, bg], F32)
            with tc.tile_pool(name="poolw", bufs=1) as ppool:
                for c in range(KH):
                    wprod = ppool.tile([128, bg, kept], BF16, tag="wprod")
                    nc.vector.tensor_tensor(
                        out=wprod[:],
                        in0=hist16[:, c, 1:, :].rearrange("p t b -> p b t"),
                        in1=attn_bc[:], op=ALU.mult)
                    nc.vector.reduce_sum(doc[:, c, :], wprod[:],
                                         axis=mybir.AxisListType.X)
            nc.sync.dma_start(doc_d[:], doc[:])
            p3w.__exit__(None, None, None)
            p3s.__exit__(None, None, None)
            ps3b.__exit__(None, None, None)
            ps3.__exit__(None, None, None)

    nc.compile()
    return nc


def _tiles(w, kc, mc):
    """w: [kc*128, mc*128] -> [mc*kc, 128, 128] lhsT tiles, m-major."""
    out = np.empty((mc * kc, 128, 128), dtype=w.dtype)
    for m in range(mc):
        for k in range(kc):
            out[m * kc + k] = w[k * 128:(k + 1) * 128, m * 128:(m + 1) * 128]
    return out


def _freeze_input(W_ih):
    """Least-norm x* with W_z x* = +40: z = sigmoid(40) == 1.0 in fp32."""
    Wz = np.asarray(W_ih[H:2 * H], np.float64)
    rhs = np.full(H, 40.0)
    x = Wz.T @ np.linalg.solve(Wz @ Wz.T, rhs)
    return x.astype(np.float32)


def host_prep(inputs, steps=LSTEPS, bg=BG):
    """Build the 8 per-core input maps (all host-side numpy)."""
    ip = np.asarray(inputs["ip"], np.float32)
    W_attn = np.asarray(inputs["W_attn"], np.float32)
    b_attn = np.asarray(inputs["b_attn"], np.float32)
    ctx = np.asarray(inputs["context"], np.float32)
    warm = WARM
    kept = steps - warm
    maps = []
    for core in range(NCORES):
        fwd = core < 4
        late = (core % 4 >= 2) if fwd else (core % 4 < 2)
        g = core % 2
        sfx = "f" if fwd else "b"
        W_ih = np.asarray(inputs[f"W_ih_{sfx}"], np.float32)
        W_hh = np.asarray(inputs[f"W_hh_{sfx}"], np.float32)
        b_ih = np.asarray(inputs[f"b_ih_{sfx}"], np.float32)
        b_hh = np.asarray(inputs[f"b_hh_{sfx}"], np.float32)

        xg = ip[g * bg:(g + 1) * bg]             # [bg, S, D]
        if not fwd:
            xg = xg[:, ::-1, :]                  # processes tau = S-1 .. 0
        # local window: early = frozen prefix + first kept half;
        # late = real warmup prefix + second kept half
        if late:
            x = xg[:, kept - warm:, :]           # [bg, warm+kept, D]
        else:
            xf = np.broadcast_to(_freeze_input(W_ih), (bg, warm, D))
            x = np.concatenate([xf, xg[:, :kept, :]], axis=1)
        assert x.shape[1] == steps

        xt = np.ascontiguousarray(x.transpose(2, 1, 0))     # [D, steps, bg]
        xt = xt.reshape(KD, 128, steps * bg)
        bias = b_ih + np.concatenate([b_hh[:2 * H], np.zeros(H, np.float32)])
        own = slice(0, H) if fwd else slice(H, 2 * H)
        pr = slice(H, 2 * H) if fwd else slice(0, H)
        bnb2 = np.zeros((128, KH, bg, 2), np.float32)
        bnb2[:, :, :, 0] = np.ascontiguousarray(
            b_hh[2 * H:].reshape(KH, 128).T)[:, :, None]
        m = {
            "xt": xt.astype(bf16),
            "wih": _tiles(W_ih.T.astype(bf16), KD, M3),
            "whh": _tiles(W_hh.T.astype(bf16), KH, M3),
            "xwb": np.ascontiguousarray(bias.reshape(M3, 128).T),
            "idn": np.eye(128, dtype=np.float32).astype(bf16),
            "bnb": bnb2.astype(bf16),
            "wao": _tiles(np.ascontiguousarray(W_attn[:, own].T).astype(bf16),
                          KH, MA),
            "wap": _tiles(np.ascontiguousarray(W_attn[:, pr].T).astype(bf16),
                          KH, MA),
            "bat": np.ascontiguousarray(b_attn.reshape(MA, 128).T),
            "ctx": np.ascontiguousarray(ctx.reshape(MA, 128).T).astype(bf16),
        }
        maps.append(m)
    return maps


def assemble(results, steps=LSTEPS, bg=BG):
    """Combine per-core partial pools: doc = sum(P)/sum(S) per half."""
    doc = np.zeros((B, 2 * H), np.float32)
    for dir_ in range(2):
        for g in range(2):
            early = g if dir_ == 0 else 6 + g
            late = 2 + g if dir_ == 0 else 4 + g
            Pe = np.asarray(results[early]["doc"]).transpose(2, 1, 0)
            Pl = np.asarray(results[late]["doc"]).transpose(2, 1, 0)
            Se = np.asarray(results[early]["ssum"])[:, 0]
            Sl = np.asarray(results[late]["ssum"])[:, 0]
            comb = (Pe + Pl).reshape(bg, H) / (Se + Sl)[:, None]
            half = slice(0, H) if dir_ == 0 else slice(H, 2 * H)
            doc[g * bg:(g + 1) * bg, half] = comb
    return doc


def kernel(**inputs):
    nc = build_program(LSTEPS, BG)
    in_maps = host_prep(inputs, LSTEPS, BG)
    res = run_bass_kernel_spmd(nc, in_maps, list(range(NCORES)))
    return assemble(res.results, LSTEPS, BG)


# revision 39
# speedup vs baseline: 1.3128x; 1.0894x over previous
"""Bidirectional-GRU document encoder (BiGRU + additive attention pooling)
for Trainium2, SPMD over 8 NeuronCores.

Sharding v2 — time-split: 8 cores = 2 directions x 2 doc-groups (16 docs)
x 2 time-halves.  Each core runs 320 local steps: 64 prefix steps + 256
kept steps covering its global-time half.  "Early" cores (which start at
the true sequence end for their direction) get an exact frozen prefix:
the host feeds warmup inputs x* solved so W_z x* + b_z ~ +40, making
z = sigmoid(40) = 1.0 exactly in fp32, so h' = z*h stays exactly 0
through the prefix.  "Late" cores warm up on the 64 real inputs
preceding their half; the h-seed error decays like prod(z) over 64
steps (~1e-3), far inside tolerance.  Serial depth drops 512 -> 320.

The per-step gate chain: 27 W_hh matmul pairs in r,z,n blocks with
separate one-bank PSUM tiles per gate (dep tracking is per-tile), a
single-matmul seed per gate from t-major xw layouts, and the
4-instruction tail sigmoid_r -> scan1 -> tanh -> scan2 via
TensorTensorScan.  q = sigmoid(-ghz) on ACT; GpSimd stays empty of
chain work so collective triggers cannot jam its queue.  Issue order is
dependency precision: consumers issue directly after their producer
block so coalesced semaphore waits snap to the true producer.

Exchange: pairwise AllGathers (fwd-half <-> bwd-core owning the same
global-time half) staged in thirds (B at kept 128, M at kept 192, A at
end) and resolved in 8-step sub-slices in the front slot of each step.
Attention scores for all kept chunks run in the tail (chunk 7 under the
final AllGather); pooling is partial (unnormalized exp) with per-core
partial sums combined on the host.
"""

import numpy as np
import ml_dtypes

import concourse.bacc as bacc
import concourse.bass as bass
import concourse.mybir as mybir
import concourse.tile as tile
from concourse.bass_utils import run_bass_kernel_spmd

F32 = mybir.dt.float32
BF16 = mybir.dt.bfloat16
AF = mybir.ActivationFunctionType
ALU = mybir.AluOpType
bf16 = ml_dtypes.bfloat16

# Problem constants
B, S, D, H = 32, 512, 768, 384
NCORES = 8
BG = 16                # docs per core
WARM = 32              # prefix steps per core
LSTEPS = S // 2 + WARM  # 320 local steps
KD = D // 128          # 6  k-chunks of input dim
M3 = 3 * H // 128      # 9  m-chunks of gate dim
KH = H // 128          # 3  k-chunks of hidden dim
MA = 2 * H // 128      # 6  m-chunks of attention rows


def build_program(steps=LSTEPS, bg=BG):
    """Build the SPMD Bass program (identical on all 8 cores)."""
    nc = bacc.Bacc("TRN2", target_bir_lowering=False, debug=False,
                   num_devices=NCORES)

    warm = WARM
    kept = steps - warm                     # 256 kept steps
    cols = steps * bg                       # P1 (t, b) plane incl. prefix
    ncol = 512                              # matmul N-chunk (one psum bank)
    pchunks = cols // ncol                  # 10 P1 chunks (local t)
    ct = ncol // bg                         # 32 timesteps per chunk
    achunks = kept * bg // ncol             # 8 attention chunks (kept t)
    qrt = kept // 4                         # 64
    half = kept // 2                        # 128

    # ---- DRAM I/O ----
    xt_d = nc.dram_tensor("xt", [KD, 128, cols], BF16, kind="ExternalInput")
    wih_d = nc.dram_tensor("wih", [M3 * KD, 128, 128], BF16, kind="ExternalInput")
    whh_d = nc.dram_tensor("whh", [M3 * KH, 128, 128], BF16, kind="ExternalInput")
    xwb_d = nc.dram_tensor("xwb", [128, M3], F32, kind="ExternalInput")
    idn_d = nc.dram_tensor("idn", [128, 128], BF16, kind="ExternalInput")
    bnb_d = nc.dram_tensor("bnb", [128, KH, bg, 2], BF16, kind="ExternalInput")
    wao_d = nc.dram_tensor("wao", [MA * KH, 128, 128], BF16, kind="ExternalInput")
    wap_d = nc.dram_tensor("wap", [MA * KH, 128, 128], BF16, kind="ExternalInput")
    bat_d = nc.dram_tensor("bat", [128, MA], F32, kind="ExternalInput")
    ctx_d = nc.dram_tensor("ctx", [128, MA], BF16, kind="ExternalInput")
    doc_d = nc.dram_tensor("doc", [128, KH, bg], F32, kind="ExternalOutput")
    ssum_d = nc.dram_tensor("ssum", [bg, 1], F32, kind="ExternalOutput")

    # Internal DRAM: hidden-state exchange staged in thirds.
    nA = qrt               # final exchange: u in [0, qrt)
    nB = kept - half       # first exchange: u in [half, kept)
    nM = half - qrt
    cc_inA = nc.dram_tensor("cc_inA", [128, nA, KH, bg], BF16)
    cc_outA = nc.dram_tensor("cc_outA", [2, 128, nA, KH, bg], BF16)
    cc_inB = nc.dram_tensor("cc_inB", [128, nB, KH, bg], BF16)
    cc_outB = nc.dram_tensor("cc_outB", [2, 128, nB, KH, bg], BF16)
    cc_inM = nc.dram_tensor("cc_inM", [128, nM, KH, bg], BF16)
    cc_outM = nc.dram_tensor("cc_outM", [2, 128, nM, KH, bg], BF16)
    sc_d = nc.dram_tensor("sc_scratch", [1, achunks, ct, bg], F32)
    at_d = nc.dram_tensor("at_scratch", [bg, kept], BF16)
    # pairs: (fwd-early-g, bwd-late-g) and (fwd-late-g, bwd-early-g)
    groups = [[0, 4], [1, 5], [2, 6], [3, 7]]

    # P1 pieces interleaved into the recurrence, globally greedy-packed:
    # each (c, m, khalf) piece lands on a free step before chunk c is
    # consumed (deadline c*ct - 2).
    # attention chunks 4-6 injected into recurrence idle windows:
    # own h ready at warm+32*(nci+1)+1; peer rows resolved by ~warm+161
    inj_base = {4: warm + 162, 5: warm + 194, 6: warm + 226}
    pieces = {}
    upfront = 1
    for c in range(upfront, pchunks):
        lo = max(0, (c - 2) * ct)
        hi = c * ct - 2
        npc = 2 * M3
        for i in range(npc):
            s = lo + (i * (hi - lo)) // npc
            pieces.setdefault(s, []).append((c, i // 2, i % 2))

    with tile.TileContext(nc) as tc:
        with (
            tc.tile_pool(name="const", bufs=1) as cpool,
            tc.tile_pool(name="state", bufs=1) as spool,
            tc.tile_pool(name="work", bufs=2) as wpool,
        ):
            # ---- constants to SBUF ----
            whh = cpool.tile([128, M3 * KH, 128], BF16)
            xwb = cpool.tile([128, M3], F32)
            idn = cpool.tile([128, 128], BF16)
            bnb2 = cpool.tile([128, KH, bg, 2], BF16)
            wao = cpool.tile([128, MA * KH, 128], BF16)
            wap = cpool.tile([128, MA * KH, 128], BF16)
            bat = cpool.tile([128, MA], F32)
            ctxt = cpool.tile([128, MA], BF16)
            nc.sync.dma_start(whh[:], whh_d[:].rearrange("t p c -> p t c"))
            nc.sync.dma_start(xwb[:], xwb_d[:])
            nc.sync.dma_start(idn[:], idn_d[:])
            nc.sync.dma_start(bnb2[:], bnb_d[:])

            # ---- persistent state ----
            hist16 = spool.tile([128, KH, kept + 1, bg], BF16)
            # gate-chain scratch: pairwise-interleaved scan operands
            sb0 = spool.tile([128, 3, bg, 2], F32)    # parity1 <- sigmoid(r)
            zbuf = spool.tile([128, KH, bg], F32)     # sigmoid(z)
            nin2 = spool.tile([128, KH, bg, 2], F32)  # scan1 out; parity1 = nin
            bufn = spool.tile([128, KH, bg, 2], F32)  # parity1 <- tanh (n)
            bufq = spool.tile([128, KH, bg, 2], F32)  # parity0 <- q, parity1 <- zh
            ring0 = spool.tile([128, KH, bg, 2], BF16)
            ring1 = spool.tile([128, KH, bg, 2], BF16)
            ring = [ring0, ring1]                     # parity1 = h(t), ping-pong
            peer_hi = spool.tile([128, kept - qrt, KH, bg], BF16)
            nc.vector.memset(sb0[:], 0.0)             # zeros at parity 0
            nc.vector.memset(bufn[:], 0.0)
            nc.vector.memset(ring[1][:], 0.0)         # h(-1) = 0

            with (
                tc.tile_pool(name="xwp", bufs=1) as xwp,
                tc.tile_pool(name="xin", bufs=1) as xpool,
                tc.tile_pool(name="xtc", bufs=2) as xtp,
                tc.tile_pool(name="ps1", bufs=2,
                             space=bass.MemorySpace.PSUM) as psA,
                tc.tile_pool(name="psr", bufs=1,
                             space=bass.MemorySpace.PSUM) as psR,
                tc.tile_pool(name="psz", bufs=1,
                             space=bass.MemorySpace.PSUM) as psZ,
                tc.tile_pool(name="psn", bufs=1,
                             space=bass.MemorySpace.PSUM) as psN,
                tc.tile_pool(name="pssc", bufs=2,
                             space=bass.MemorySpace.PSUM) as psSC,
                tc.tile_pool(name="pscm", bufs=1,
                             space=bass.MemorySpace.PSUM) as psCM,
            ):
                # input projections, seed-friendly t-major layouts
                xws = xwp.tile([128, steps, 6, bg], BF16)
                xwn = xwp.tile([128, steps, KH, bg], BF16)
                wih = xpool.tile([128, M3 * KD, 128], BF16)
                nc.sync.dma_start(wih[:], wih_d[:].rearrange("t p c -> p t c"))

                xtc_tiles = {}
                px_tiles = {}
                pending_bias = []

                def xw_fetch(c):
                    csl = slice(c * ncol, (c + 1) * ncol)
                    xtc = xtp.tile([128, KD, ncol], BF16, tag="xtc")
                    for k in range(KD):
                        nc.sync.dma_start(xtc[:, k, :], xt_d[k][:, csl])
                    xtc_tiles[c] = xtc

                def xw_bias(c, m):
                    px = px_tiles.pop((c, m))
                    dst = (xws[:, c * ct:(c + 1) * ct, m, :] if m < 6
                           else xwn[:, c * ct:(c + 1) * ct, m - 6, :])
                    nc.vector.tensor_scalar(
                        out=dst,
                        in0=px[:].rearrange("p (t b) -> p t b", b=bg),
                        scalar1=xwb[:, m:m + 1],
                        scalar2=None, op0=ALU.add)

                def xw_piece(c, m, ks, defer=False):
                    if c not in xtc_tiles:
                        xw_fetch(c)
                    xtc = xtc_tiles[c]
                    if ks[0] == 0:
                        px = psA.tile([128, ncol], F32, tag="px")
                        px_tiles[(c, m)] = px
                    px = px_tiles[(c, m)]
                    for k in ks:
                        nc.tensor.matmul(
                            px[:], wih[:, m * KD + k, :], xtc[:, k, :],
                            start=(k == 0), stop=(k == KD - 1))
                    if ks[-1] == KD - 1:
                        if defer:
                            pending_bias.append((c, m))
                        else:
                            xw_bias(c, m)

                # Phase 1 prologue: chunk 0 so the recurrence can start
                for c in range(upfront):
                    for m in range(M3):
                        xw_piece(c, m, [0, 1, 2])
                        xw_piece(c, m, [3, 4, 5])

                # ======= Phase 2: GRU recurrence =======
                def seed(t):
                    gr = psR.tile([128, 3, bg], F32, tag="gr")
                    gz = psZ.tile([128, 3, bg], F32, tag="gz")
                    gn2 = psN.tile([128, KH, bg, 2], F32, tag="gn2")
                    nc.tensor.matmul(gr[:], idn[:], xws[:, t, 0:3, :],
                                     start=True, stop=False)
                    nc.tensor.matmul(gz[:], idn[:], xws[:, t, 3:6, :],
                                     start=True, stop=False)
                    nc.tensor.matmul(gn2[:], idn[:], bnb2[:],
                                     start=True, stop=False)
                    nc.tensor.matmul(gn2[:, :, :, 1], idn[:],
                                     xwn[:, t, :, :],
                                     start=False, stop=False)
                    return gr, gz, gn2

                RS = 4
                def resolve_slice(cout, cin, j0, w, j):
                    lo = w * ct + j * RS
                    sl = slice(lo, lo + RS)
                    pslice = peer_hi[:, j0 + lo:j0 + lo + RS, :, :]
                    rs1 = wpool.tile([128, RS, KH, bg], BF16, tag="rs1")
                    rso = wpool.tile([128, RS, KH, bg], BF16, tag="rso")
                    nc.sync.dma_start(pslice, cout[0][:, sl, :, :])
                    nc.sync.dma_start(rs1[:], cout[1][:, sl, :, :])
                    nc.sync.dma_start(rso[:], cin[:, sl, :, :])
                    nc.vector.tensor_tensor(out=pslice, in0=pslice,
                                            in1=rs1[:], op=ALU.add)
                    nc.vector.tensor_tensor(out=pslice, in0=pslice,
                                            in1=rso[:], op=ALU.subtract)

                front = {}
                spw = ct // RS
                for w in range(nB // ct):
                    for j in range(spw):
                        front.setdefault(warm + half + 17 + spw * w + j,
                                         []).append(
                            (cc_outB, cc_inB, half - qrt, w, j))
                for w in range(nM // ct):
                    for j in range(spw):
                        front.setdefault(warm + 3 * qrt + 17 + spw * w + j,
                                         []).append(
                            (cc_outM, cc_inM, 0, w, j))

                inj_state = {}

                def inj_own(nci, m):
                    pa = psSC.tile([128, ncol], F32, tag="spa")
                    inj_state[(nci, m)] = pa
                    if m == 0:
                        psc = psCM.tile([1, ncol], F32, tag="pscm")
                        inj_state[nci] = psc
                    for k in range(KH):
                        nc.tensor.matmul(
                            pa[:], wao[:, m * KH + k, :],
                            hist16[:, k, 1 + nci * ct:1 + (nci + 1) * ct, :],
                            start=(k == 0), stop=False)

                def inj_peer(nci, m):
                    pa = inj_state[(nci, m)]
                    s0 = nci * ct - qrt
                    for k in range(KH):
                        nc.tensor.matmul(
                            pa[:], wap[:, m * KH + k, :],
                            peer_hi[:, s0:s0 + ct, k, :],
                            start=False, stop=(k == KH - 1))

                def inj_tanh(nci, m):
                    pa = inj_state.pop((nci, m))
                    psc = inj_state[nci]
                    th = wpool.tile([128, ncol], BF16, tag="ith")
                    nc.scalar.activation(th[:], pa[:], AF.Tanh,
                                         bias=bat[:, m:m + 1])
                    nc.tensor.matmul(psc[:], ctxt[:, m:m + 1], th[:],
                                     start=(m == 0), stop=(m == MA - 1))

                def inj_done(nci):
                    psc = inj_state.pop(nci)
                    scev = wpool.tile([1, ncol], F32, tag="iscev")
                    nc.vector.tensor_copy(scev[:], psc[:])
                    nc.sync.dma_start(
                        sc_d[0, nci].unsqueeze(0),
                        scev[:].rearrange("o (t b) -> o t b", t=ct))

                inject = {}
                for nci, base in inj_base.items():
                    for m in range(MA):
                        inject.setdefault(base + 3 * m, []).append(
                            (inj_own, (nci, m)))
                        inject.setdefault(base + 3 * m + 1, []).append(
                            (inj_peer, (nci, m)))
                        inject.setdefault(base + 3 * m + 2, []).append(
                            (inj_tanh, (nci, m)))
                    inject.setdefault(base + 3 * MA, []).append(
                        (inj_done, (nci,)))

                # attention weights are tail-only: load them behind the
                # recurrence, off the critical prologue DMA window
                nc.sync.dma_start(wao[:], wao_d[:].rearrange("t p c -> p t c"))
                nc.sync.dma_start(wap[:], wap_d[:].rearrange("t p c -> p t c"))
                nc.sync.dma_start(bat[:], bat_d[:])
                nc.sync.dma_start(ctxt[:], ctx_d[:])

                nxt = seed(0)
                for t in range(steps):
                    rcur = ring[t % 2]
                    rprev = ring[(t + 1) % 2]
                    gr, gz, gn2 = nxt
                    for m in (0, 1, 2):
                        for k in range(KH):
                            nc.tensor.matmul(
                                gr[:, m, :], whh[:, m * KH + k, :],
                                rprev[:, k, :, 1], start=False,
                                stop=(k == KH - 1 and m == 2))
                    # ---- front slot: previous step's bookkeeping ----
                    while pending_bias:
                        xw_bias(*pending_bias.pop(0))
                    if t > warm:
                        kc = t - warm          # hist col for h(t-1)
                        nc.vector.tensor_copy(hist16[:, :, kc, :],
                                              rprev[:, :, :, 1])
                        u = kept - kc          # = kept-1 - (kc-1)
                        if u >= half:
                            nc.sync.dma_start(cc_inB[:, u - half, :, :],
                                              hist16[:, :, kc, :])
                        elif u >= qrt:
                            nc.sync.dma_start(cc_inM[:, u - qrt, :, :],
                                              hist16[:, :, kc, :])
                        else:
                            nc.sync.dma_start(cc_inA[:, u, :, :],
                                              hist16[:, :, kc, :])
                    for args in front.get(t, ()):
                        resolve_slice(*args)
                    if t == warm + half:
                        nc.gpsimd.collective_compute(
                            "AllGather", ALU.bypass, replica_groups=groups,
                            ins=[cc_inB[:]], outs=[cc_outB[:]])
                    if t == warm + 3 * qrt:
                        nc.gpsimd.collective_compute(
                            "AllGather", ALU.bypass, replica_groups=groups,
                            ins=[cc_inM[:]], outs=[cc_outM[:]])
                    # r = sigmoid(ghr): waits only on the r-stop
                    nc.scalar.activation(sb0[:, :, :, 1], gr[:], AF.Sigmoid)
                    for m in (3, 4, 5):
                        for k in range(KH):
                            nc.tensor.matmul(
                                gz[:, m - 3, :], whh[:, m * KH + k, :],
                                rprev[:, k, :, 1], start=False,
                                stop=(k == KH - 1 and m == 5))
                    # z and q = sigmoid(+-ghz) on ACT; GpSimd stays empty
                    nc.scalar.activation(zbuf[:], gz[:], AF.Sigmoid)
                    nc.scalar.activation(bufq[:, :, :, 0], gz[:],
                                         AF.Sigmoid, scale=-1.0)
                    for m in (6, 7, 8):
                        for k in range(KH):
                            nc.tensor.matmul(
                                gn2[:, m - 6, :, 0], whh[:, m * KH + k, :],
                                rprev[:, k, :, 1], start=False,
                                stop=(k == KH - 1 and m == 8))
                    # scan1: even -> ghn, odd -> r*ghn + xn  (= nin)
                    nc.vector.tensor_tensor_scan(
                        out=nin2[:].rearrange("p c b j -> p (c b j)"),
                        data0=sb0[:].rearrange("p c b j -> p (c b j)"),
                        data1=gn2[:].rearrange("p c b j -> p (c b j)"),
                        initial=0.0, op0=ALU.mult, op1=ALU.add)
                    # zh = z * h(t-1): DVE, after scan1, before tanh's issue
                    nc.vector.tensor_tensor(
                        out=bufq[:, :, :, 1], in0=zbuf[:],
                        in1=rprev[:, :, :, 1], op=ALU.mult)
                    if t + 1 < steps:
                        nxt = seed(t + 1)
                    # n = tanh(nin) -> parity-1 of bufn
                    nc.scalar.activation(bufn[:, :, :, 1],
                                         nin2[:, :, :, 1], AF.Tanh)
                    # PE fill work in the idle window
                    for (c, m, kh) in pieces.get(t, ()):
                        xw_piece(c, m, [0, 1, 2] if kh == 0 else [3, 4, 5],
                                 defer=True)
                    for fn, args in inject.get(t, ()):
                        if fn in (inj_own, inj_peer):
                            fn(*args)
                    # scan2: even -> q, odd -> n*q + zh  (= h')
                    nc.vector.tensor_tensor_scan(
                        out=rcur[:].rearrange("p c b j -> p (c b j)"),
                        data0=bufn[:].rearrange("p c b j -> p (c b j)"),
                        data1=bufq[:].rearrange("p c b j -> p (c b j)"),
                        initial=0.0, op0=ALU.mult, op1=ALU.add)
                    for fn, args in inject.get(t, ()):
                        if fn not in (inj_own, inj_peer):
                            fn(*args)

                # flush the last kept step
                rlast = ring[(steps - 1) % 2]
                nc.vector.tensor_copy(hist16[:, :, kept, :],
                                      rlast[:, :, :, 1])
                nc.sync.dma_start(cc_inA[:, 0, :, :], hist16[:, :, kept, :])

            # ======= Phase 3: exchange + attention + partial pooling =======
            ps3 = tc.tile_pool(name="ps3", bufs=1, space=bass.MemorySpace.PSUM)
            psA3 = ps3.__enter__()
            ps3b = tc.tile_pool(name="ps3b", bufs=2,
                                space=bass.MemorySpace.PSUM)
            psB3 = ps3b.__enter__()
            p3s = tc.tile_pool(name="p3s", bufs=1)
            spool3 = p3s.__enter__()
            p3w = tc.tile_pool(name="p3w", bufs=1)
            wpool3 = p3w.__enter__()

            nc.gpsimd.collective_compute(
                "AllGather", ALU.bypass, replica_groups=groups,
                ins=[cc_inA[:]], outs=[cc_outA[:]])
            peer_lo = spool3.tile([128, nA, KH, bg], BF16)

            own_pas = {}

            def chunk_own(nci):
                pas = []
                for m in range(MA):
                    pa = psA3.tile([128, ncol], F32, tag=f"pa{m}")
                    for k in range(KH):
                        nc.tensor.matmul(
                            pa[:], wao[:, m * KH + k, :],
                            hist16[:, k, 1 + nci * ct:1 + (nci + 1) * ct, :],
                            start=(k == 0), stop=False)
                    pas.append(pa)
                own_pas[nci] = pas

            def chunk_scores(nci, ptile, soff):
                tsl = slice(nci * ct - soff, (nci + 1) * ct - soff)
                psc = psB3.tile([1, ncol], F32, tag="psc")
                if nci not in own_pas:
                    chunk_own(nci)
                pas = own_pas.pop(nci)
                for m in range(MA):
                    for k in range(KH):
                        nc.tensor.matmul(
                            pas[m][:], wap[:, m * KH + k, :],
                            ptile[:, tsl, k, :],
                            start=False, stop=(k == KH - 1))
                ths = []
                for m in range(MA):
                    th = wpool3.tile([128, ncol], BF16, tag=f"th{m}")
                    nc.scalar.activation(th[:], pas[m][:], AF.Tanh,
                                         bias=bat[:, m:m + 1])
                    ths.append(th)
                for m in range(MA):
                    nc.tensor.matmul(psc[:], ctxt[:, m:m + 1], ths[m][:],
                                     start=(m == 0), stop=(m == MA - 1))
                scev = wpool3.tile([1, ncol], F32, tag="scev")
                nc.vector.tensor_copy(scev[:], psc[:])
                nc.sync.dma_start(
                    sc_d[0, nci].unsqueeze(0),
                    scev[:].rearrange("o (t b) -> o t b", t=ct))

            # chunk 7 + remaining hi chunks run during AllGather A
            for nci in range(2, achunks):
                if nci not in inj_base:
                    chunk_scores(nci, peer_hi, qrt)

            def resolve_lo_slice(w):
                sl = slice(w * ct, (w + 1) * ct)
                pslice = peer_lo[:, sl, :, :]
                s1t = wpool3.tile([128, ct, KH, bg], BF16, tag="s1")
                ownr = wpool3.tile([128, ct, KH, bg], BF16, tag="ownr")
                nc.sync.dma_start(pslice, cc_outA[0][:, sl, :, :])
                nc.sync.dma_start(s1t[:], cc_outA[1][:, sl, :, :])
                nc.sync.dma_start(ownr[:], cc_inA[:, sl, :, :])
                nc.vector.tensor_tensor(out=pslice, in0=pslice, in1=s1t[:],
                                        op=ALU.add)
                nc.vector.tensor_tensor(out=pslice, in0=pslice, in1=ownr[:],
                                        op=ALU.subtract)

            for w, nci in enumerate(range(2)):
                chunk_own(nci)       # own-half mms hide the resolve
                resolve_lo_slice(w)
                chunk_scores(nci, peer_lo, 0)

            # scores -> [bg, kept] via DRAM; unnormalized softmax weights
            sc = spool3.tile([bg, kept], F32)
            nc.sync.dma_start(sc[:].rearrange("b (n t) -> b n t", n=achunks),
                              sc_d[0].rearrange("n t b -> b n t"))
            esc = wpool3.tile([bg, kept], F32, tag="esc")
            ssum = wpool3.tile([bg, 1], F32, tag="ssum")
            nc.scalar.activation(esc[:], sc[:], AF.Exp, accum_out=ssum[:])
            nc.sync.dma_start(ssum_d[:], ssum[:])
            attn = spool3.tile([bg, kept], BF16)
            nc.vector.tensor_copy(attn[:], esc[:])
            # broadcast attn to all partitions as [128, (b, t)] via DRAM
            nc.sync.dma_start(at_d[:], attn[:])
            attn_bc = spool3.tile([128, bg, kept], BF16)
            nc.sync.dma_start(attn_bc[:],
                              at_d[:].unsqueeze(0).broadcast_to(
                                  [128, bg, kept]))

            # partial pooling: P[p, c, b] = sum_t h[p, c, t, b] * e[b, t]
            doc = spool3.tile([128, KH, bg], F32)
            with tc.tile_pool(name="poolw", bufs=1) as ppool:
                for c in range(KH):
                    wprod = ppool.tile([128, bg, kept], BF16, tag="wprod")
                    nc.vector.tensor_tensor(
                        out=wprod[:],
                        in0=hist16[:, c, 1:, :].rearrange("p t b -> p b t"),
                        in1=attn_bc[:], op=ALU.mult)
                    nc.vector.reduce_sum(doc[:, c, :], wprod[:],
                                         axis=mybir.AxisListType.X)
            nc.sync.dma_start(doc_d[:], doc[:])
            p3w.__exit__(None, None, None)
            p3s.__exit__(None, None, None)
            ps3b.__exit__(None, None, None)
            ps3.__exit__(None, None, None)

    nc.compile()
    return nc


def _tiles(w, kc, mc):
    """w: [kc*128, mc*128] -> [mc*kc, 128, 128] lhsT tiles, m-major."""
    out = np.empty((mc * kc, 128, 128), dtype=w.dtype)
    for m in range(mc):
        for k in range(kc):
            out[m * kc + k] = w[k * 128:(k + 1) * 128, m * 128:(m + 1) * 128]
    return out


def _freeze_input(W_ih):
    """Least-norm x* with W_z x* = +40: z = sigmoid(40) == 1.0 in fp32."""
    Wz = np.asarray(W_ih[H:2 * H], np.float64)
    rhs = np.full(H, 40.0)
    x = Wz.T @ np.linalg.solve(Wz @ Wz.T, rhs)
    return x.astype(np.float32)


def host_prep(inputs, steps=LSTEPS, bg=BG):
    """Build the 8 per-core input maps (all host-side numpy)."""
    ip = np.asarray(inputs["ip"], np.float32)
    W_attn = np.asarray(inputs["W_attn"], np.float32)
    b_attn = np.asarray(inputs["b_attn"], np.float32)
    ctx = np.asarray(inputs["context"], np.float32)
    warm = WARM
    kept = steps - warm
    maps = []
    for core in range(NCORES):
        fwd = core < 4
        late = (core % 4 >= 2) if fwd else (core % 4 < 2)
        g = core % 2
        sfx = "f" if fwd else "b"
        W_ih = np.asarray(inputs[f"W_ih_{sfx}"], np.float32)
        W_hh = np.asarray(inputs[f"W_hh_{sfx}"], np.float32)
        b_ih = np.asarray(inputs[f"b_ih_{sfx}"], np.float32)
        b_hh = np.asarray(inputs[f"b_hh_{sfx}"], np.float32)

        xg = ip[g * bg:(g + 1) * bg]             # [bg, S, D]
        if not fwd:
            xg = xg[:, ::-1, :]                  # processes tau = S-1 .. 0
        # local window: early = frozen prefix + first kept half;
        # late = real warmup prefix + second kept half
        if late:
            x = xg[:, kept - warm:, :]           # [bg, warm+kept, D]
        else:
            xf = np.broadcast_to(_freeze_input(W_ih), (bg, warm, D))
            x = np.concatenate([xf, xg[:, :kept, :]], axis=1)
        assert x.shape[1] == steps

        xt = np.ascontiguousarray(x.transpose(2, 1, 0))     # [D, steps, bg]
        xt = xt.reshape(KD, 128, steps * bg)
        bias = b_ih + np.concatenate([b_hh[:2 * H], np.zeros(H, np.float32)])
        own = slice(0, H) if fwd else slice(H, 2 * H)
        pr = slice(H, 2 * H) if fwd else slice(0, H)
        bnb2 = np.zeros((128, KH, bg, 2), np.float32)
        bnb2[:, :, :, 0] = np.ascontiguousarray(
            b_hh[2 * H:].reshape(KH, 128).T)[:, :, None]
        m = {
            "xt": xt.astype(bf16),
            "wih": _tiles(W_ih.T.astype(bf16), KD, M3),
            "whh": _tiles(W_hh.T.astype(bf16), KH, M3),
            "xwb": np.ascontiguousarray(bias.reshape(M3, 128).T),
            "idn": np.eye(128, dtype=np.float32).astype(bf16),
            "bnb": bnb2.astype(bf16),
            "wao": _tiles(np.ascontiguousarray(W_attn[:, own].T).astype(bf16),
                          KH, MA),
            "wap": _tiles(np.ascontiguousarray(W_attn[:, pr].T).astype(bf16),
                          KH, MA),
            "bat": np.ascontiguousarray(b_attn.reshape(MA, 128).T),
            "ctx": np.ascontiguousarray(ctx.reshape(MA, 128).T).astype(bf16),
        }
        maps.append(m)
    return maps


def assemble(results, steps=LSTEPS, bg=BG):
    """Combine per-core partial pools: doc = sum(P)/sum(S) per half."""
    doc = np.zeros((B, 2 * H), np.float32)
    for dir_ in range(2):
        for g in range(2):
            early = g if dir_ == 0 else 6 + g
            late = 2 + g if dir_ == 0 else 4 + g
            Pe = np.asarray(results[early]["doc"]).transpose(2, 1, 0)
            Pl = np.asarray(results[late]["doc"]).transpose(2, 1, 0)
            Se = np.asarray(results[early]["ssum"])[:, 0]
            Sl = np.asarray(results[late]["ssum"])[:, 0]
            comb = (Pe + Pl).reshape(bg, H) / (Se + Sl)[:, None]
            half = slice(0, H) if dir_ == 0 else slice(H, 2 * H)
            doc[g * bg:(g + 1) * bg, half] = comb
    return doc


def kernel(**inputs):
    nc = build_program(LSTEPS, BG)
    in_maps = host_prep(inputs, LSTEPS, BG)
    res = run_bass_kernel_spmd(nc, in_maps, list(range(NCORES)))
    return assemble(res.results, LSTEPS, BG)
